# revision 11
# baseline (speedup 1.0000x reference)
"""Trainium2 Bass kernel for the dual-softmax cross-attention module.

Sharding: 8 cores = batch (4) x head-half (2).  Core c handles batch c//2 and
heads 4*(c%2) .. 4*(c%2)+4.  Each core computes Q/K/V projections for its
head-group, the 2048x2048 score matrix per head, one shared E = exp(s/8)
(both softmaxes are shift-invariant; scores are O(1) so no max subtraction),
contexts for both streams, exchanges context halves with its pair core via a
2-core AllGather, and produces a disjoint 256-channel slice of both outputs.

All matmuls run in bf16 (fp32 PSUM accumulation); residual + output stay fp32.
"""

import sys

for _p in ("/opt/trn_rl_repo", "/opt/pypackages"):
    if _p not in sys.path:
        sys.path.insert(0, _p)

import numpy as np
import ml_dtypes

import concourse.bass as bass
import concourse.tile as tile
from concourse import bacc, mybir
from concourse.bass_utils import run_bass_kernel_spmd

F32 = mybir.dt.float32
BF16 = mybir.dt.bfloat16
AF = mybir.ActivationFunctionType
AX = mybir.AxisListType

N_CORES = 8
B = 4          # batch
C = 512        # channels
N = 2048       # tokens (8*16*16)
H = 8          # heads
DH = 64        # head dim
HL = 4         # heads per core
CL = 256       # channels per core (head-group)
NT = N // 128  # 16 token tiles
CT = C // 128  # 4 channel tiles

_BF = ml_dtypes.bfloat16


def _build():
    nc = bacc.Bacc("TRN2", target_bir_lowering=False, debug=False,
                   num_devices=N_CORES)

    def din(name, shape, dt=BF16):
        return nc.dram_tensor(name, shape, dt, kind="ExternalInput").ap()

    x1b = din("x1b", [CT, 128, N])          # x1[b] channel-major, bf16
    x2b = din("x2b", [CT, 128, N])
    wq = din("wq", [CT, 128, CL])           # column slice of Wq
    wk = din("wk", [CT, 128, CL])
    wv1 = din("wv1", [CT, 128, CL])
    wv2 = din("wv2", [CT, 128, CL])
    wo1 = din("wo1", [CT, 128, CL])         # Wo columns for my output rows
    wo2 = din("wo2", [CT, 128, CL])
    bq = din("bq", [2, 128, 1], F32)        # bias slices per M-tile
    bk = din("bk", [2, 128, 1], F32)
    bv1 = din("bv1", [1, CL])
    bv2 = din("bv2", [1, CL])
    x1r = din("x1r", [2, 128, N], F32)      # x1[b] residual slice + bo1
    x2r = din("x2r", [2, 128, N], F32)

    o1 = nc.dram_tensor("o1", [2, 128, N], F32, kind="ExternalOutput").ap()
    o2 = nc.dram_tensor("o2", [2, 128, N], F32, kind="ExternalOutput").ap()

    with tile.TileContext(nc) as tc:
        _emit(nc, tc, locals())
    nc.compile()
    return nc


def _emit(nc, tc, t):
    x1b, x2b = t["x1b"], t["x2b"]
    wq, wk, wv1, wv2 = t["wq"], t["wk"], t["wv1"], t["wv2"]
    wo1, wo2 = t["wo1"], t["wo2"]
    bq, bk, bv1, bv2 = t["bq"], t["bk"], t["bv1"], t["bv2"]
    x1r, x2r, o1, o2 = t["x1r"], t["x2r"], t["o1"], t["o2"]

    from contextlib import ExitStack
    ctx = ExitStack()
    with ctx:
        persist = ctx.enter_context(tc.tile_pool(name="persist", bufs=1))
        small = ctx.enter_context(tc.tile_pool(name="small", bufs=8))
        vp_pool = ctx.enter_context(tc.tile_pool(name="vp", bufs=4))
        dram = ctx.enter_context(tc.tile_pool(name="dram", bufs=2, space="DRAM"))

        # ---- persistent SBUF tensors ----
        wq_s = persist.tile([128, CT, CL], BF16, tag="wq")
        wk_s = persist.tile([128, CT, CL], BF16, tag="wk")
        wv1_s = persist.tile([128, CT, CL], BF16, tag="wv1")
        wv2_s = persist.tile([128, CT, CL], BF16, tag="wv2")
        wo1_s = persist.tile([128, CT, CL], BF16, tag="wo1")
        wo2_s = persist.tile([128, CT, CL], BF16, tag="wo2")
        bq_s = persist.tile([128, 2, 1], F32, tag="bq")
        bk_s = persist.tile([128, 2, 1], F32, tag="bk")
        bv1_s = persist.tile([1, CL], BF16, tag="bv1")
        bv2_s = persist.tile([1, CL], BF16, tag="bv2")
        ones_s = persist.tile([1, N], BF16, tag="ones")
        qt_s = persist.tile([128, 2, N], BF16, tag="qt")    # Q^T  (chan-major)
        kt_s = persist.tile([128, 2, N], BF16, tag="kt")    # K^T
        v1tok = persist.tile([128, NT, CL], BF16, tag="v1tok")  # token-major V1
        v2tok = persist.tile([128, NT, CL], BF16, tag="v2tok")
        et_buf = persist.tile([128, NT, N], BF16, tag="et")     # E^T of one head
        ctxm1 = persist.tile([128, CT, N], BF16, tag="ctxm1")   # gathered ctx^T
        ctxm2 = persist.tile([128, CT, N], BF16, tag="ctxm2")

        for dst, src in ((wq_s, wq), (wk_s, wk), (wv1_s, wv1), (wv2_s, wv2),
                         (wo1_s, wo1), (wo2_s, wo2)):
            nc.sync.dma_start(dst[:, :, :], src.rearrange("t p c -> p t c"))
        nc.sync.dma_start(bq_s[:, :, :], bq.rearrange("t p c -> p t c"))
        nc.sync.dma_start(bk_s[:, :, :], bk.rearrange("t p c -> p t c"))
        nc.sync.dma_start(bv1_s[:, :], bv1[:, :])
        nc.sync.dma_start(bv2_s[:, :], bv2[:, :])
        nc.vector.memset(ones_s[:, :], 1.0)

        # ---- P1: projections ----
        # x1 tiles -> Q (chan-major) and V1 (token-major); then x2 -> K, V2.
        p1 = ExitStack()
        pj_ps = p1.enter_context(tc.tile_pool(name="pj_ps", bufs=2, space="PSUM"))
        xb_pool = p1.enter_context(tc.tile_pool(name="xb", bufs=4))
        for xb_dram, w_qk, b_qk, qk_dst, w_v, b_v, v_dst in (
            (x1b, wq_s, bq_s, qt_s, wv1_s, bv1_s, v1tok),
            (x2b, wk_s, bk_s, kt_s, wv2_s, bv2_s, v2tok),
        ):
            xt = [xb_pool.tile([128, N], BF16, tag="xb", name=f"xt{i}")
                  for i in range(CT)]
            for ti in range(CT):
                nc.sync.dma_start(xt[ti][:, :], xb_dram[ti, :, :])
            # chan-major Q/K:  out[cl, n] = sum_cin W[cin, cl] * x[cin, n]
            for m in range(2):
                ps = pj_ps.tile([128, N], F32, tag="pj")
                for ch in range(4):
                    for ti in range(CT):
                        nc.tensor.matmul(
                            ps[:, ch * 512:(ch + 1) * 512],
                            w_qk[:, ti, m * 128:(m + 1) * 128],
                            xt[ti][:, ch * 512:(ch + 1) * 512],
                            start=(ti == 0), stop=(ti == CT - 1))
                nc.scalar.activation(qk_dst[:, m, :], ps[:, :], AF.Identity,
                                     bias=b_qk[:, m, :])
            # token-major V:  out[n, cl] = sum_cin x[cin, n] * W[cin, cl] + bv
            for nt in range(NT):
                ps = pj_ps.tile([128, CL], F32, tag="pj")
                for ti in range(CT):
                    nc.tensor.matmul(
                        ps[:, :], xt[ti][:, nt * 128:(nt + 1) * 128],
                        w_v[:, ti, :], start=(ti == 0), stop=False)
                nc.tensor.matmul(ps[:, :], ones_s[:, nt * 128:(nt + 1) * 128],
                                 b_v[:, :], start=False, stop=True)
                nc.scalar.activation(v_dst[:, nt, :], ps[:, :], AF.Copy)
        p1.close()

        # ---- P2: per-head attention ----
        p2 = ExitStack()
        sc_ps = p2.enter_context(tc.tile_pool(name="sc_ps", bufs=2, space="PSUM"))
        ctx_ps = p2.enter_context(tc.tile_pool(name="ctx_ps", bufs=1, space="PSUM"))
        eslab = p2.enter_context(tc.tile_pool(name="eslab", bufs=2))
        gsrc_pool = p2.enter_context(tc.tile_pool(name="gsrc", bufs=2))
        for hl in range(HL):
            g, poff = hl // 2, 64 * (hl % 2)
            q_l = qt_s[poff:poff + 64, g, :]
            k_l = kt_s[poff:poff + 64, g, :]
            cps = ctx_ps.tile([128, N], F32, tag="ctx")   # rows 0:64 ctx1, 64:128 ctx2
            cs_t = small.tile([128, NT], F32, tag="cs")
            for qt in range(NT):
                sps = sc_ps.tile([128, 1024], F32, tag="sc")
                sps2 = sc_ps.tile([128, 1024], F32, tag="sc")
                es = eslab.tile([128, N], BF16, tag="es")
                rs_p = small.tile([128, 2], F32, tag="rsp")
                for u, ps in enumerate((sps, sps2)):
                    for ch in range(2):
                        off = u * 1024 + ch * 512
                        nc.tensor.matmul(
                            ps[:, ch * 512:(ch + 1) * 512],
                            q_l[:, qt * 128:(qt + 1) * 128],
                            k_l[:, off:off + 512],
                            start=True, stop=True)
                    nc.scalar.activation(es[:, u * 1024:(u + 1) * 1024], ps[:, :],
                                         AF.Exp, scale=0.125,
                                         accum_out=rs_p[:, u:u + 1])
                rs = small.tile([128, 1], F32, tag="rs")
                nc.vector.tensor_add(rs[:, :], rs_p[:, 0:1], rs_p[:, 1:2])
                rr = small.tile([128, 1], F32, tag="rr")
                nc.vector.reciprocal(rr[:, :], rs[:, :])
                v2p = vp_pool.tile([128, DH], BF16, tag="v2p")
                nc.vector.tensor_scalar_mul(
                    v2p[:, :], v2tok[:, qt, hl * DH:(hl + 1) * DH], rr[:, :])
                for ch in range(4):
                    nc.tensor.matmul(
                        cps[64:128, ch * 512:(ch + 1) * 512],
                        v2p[:, :], es[:, ch * 512:(ch + 1) * 512],
                        start=(qt == 0), stop=(qt == NT - 1))
                # transpose E-slab into E^T buffer (bf16 xbar transpose)
                nc.sync.dma_start(et_buf[:, :, qt * 128:(qt + 1) * 128],
                                  es[:, :], transpose=True)
            # colsum over q (free dim of E^T), reciprocal, ctx1
            for kt in range(NT):
                nc.vector.reduce_sum(out=cs_t[:, kt:kt + 1],
                                     in_=et_buf[:, kt, :], axis=AX.X)
            cr_t = small.tile([128, NT], F32, tag="cr")
            nc.vector.reciprocal(cr_t[:, :], cs_t[:, :])
            for kt in range(NT):
                v1p = vp_pool.tile([128, DH], BF16, tag="v1p")
                nc.vector.tensor_scalar_mul(
                    v1p[:, :], v1tok[:, kt, hl * DH:(hl + 1) * DH],
                    cr_t[:, kt:kt + 1])
                for ch in range(4):
                    nc.tensor.matmul(
                        cps[0:64, ch * 512:(ch + 1) * 512],
                        v1p[:, :], et_buf[:, kt, ch * 512:(ch + 1) * 512],
                        start=(kt == 0), stop=(kt == NT - 1))
            # evacuate both contexts, gather the head pair across the 2 cores
            gs = gsrc_pool.tile([128, N], BF16, tag="gs")
            nc.vector.tensor_copy(gs[0:64, :], cps[0:64, :])
            nc.vector.tensor_copy(gs[64:128, :], cps[64:128, :])
            gin = dram.tile([128, N], BF16, tag="gin")
            gout = dram.tile([2, 128, N], BF16, tag="gout")
            nc.sync.dma_start(gin[:, :], gs[:, :])
            nc.gpsimd.collective_compute(
                "AllGather", mybir.AluOpType.bypass,
                replica_groups=[[0, 1], [2, 3], [4, 5], [6, 7]],
                ins=[gin.opt()], outs=[gout.opt()])
            for r in range(2):
                tt = 2 * r + hl // 2
                nc.sync.dma_start(ctxm1[poff:poff + 64, tt, :], gout[r, 0:64, :])
                nc.sync.dma_start(ctxm2[poff:poff + 64, tt, :], gout[r, 64:128, :])

        p2.close()

        # ---- P3: output projections + residual ----
        p3 = ExitStack()
        o_ps = p3.enter_context(tc.tile_pool(name="o_ps", bufs=2, space="PSUM"))
        xr_pool = p3.enter_context(tc.tile_pool(name="xr", bufs=2))
        out_pool = p3.enter_context(tc.tile_pool(name="outp", bufs=2))
        for w_s, cm, xr, oo in ((wo1_s, ctxm1, x1r, o1), (wo2_s, ctxm2, x2r, o2)):
            for m in range(2):
                xr_t = xr_pool.tile([128, N], F32, tag="xr")
                nc.sync.dma_start(xr_t[:, :], xr[m, :, :])
                ps = o_ps.tile([128, N], F32, tag="o")
                for ch in range(4):
                    for ti in range(CT):
                        nc.tensor.matmul(
                            ps[:, ch * 512:(ch + 1) * 512],
                            w_s[:, ti, m * 128:(m + 1) * 128],
                            cm[:, ti, ch * 512:(ch + 1) * 512],
                            start=(ti == 0), stop=(ti == CT - 1))
                ot = out_pool.tile([128, N], F32, tag="ot")
                nc.vector.tensor_add(ot[:, :], ps[:, :], xr_t[:, :])
                nc.sync.dma_start(oo[m, :, :], ot[:, :])
        p3.close()


_NC_CACHE = None


def _get_nc():
    global _NC_CACHE
    if _NC_CACHE is None:
        _NC_CACHE = _build()
    return _NC_CACHE


def _in_maps(x1, x2, Wq, bq, Wk, bk, Wv1, bv1, Wv2, bv2, Wo1, bo1, Wo2, bo2):
    x1f = np.asarray(x1, np.float32).reshape(B, C, N)
    x2f = np.asarray(x2, np.float32).reshape(B, C, N)
    in_maps = []
    for c in range(N_CORES):
        b, hq = c // 2, c % 2
        sl = slice(CL * hq, CL * hq + CL)
        m = {
            "x1b": x1f[b].reshape(CT, 128, N).astype(_BF),
            "x2b": x2f[b].reshape(CT, 128, N).astype(_BF),
            "wq": np.asarray(Wq, np.float32)[:, sl].reshape(CT, 128, CL).astype(_BF),
            "wk": np.asarray(Wk, np.float32)[:, sl].reshape(CT, 128, CL).astype(_BF),
            "wv1": np.asarray(Wv1, np.float32)[:, sl].reshape(CT, 128, CL).astype(_BF),
            "wv2": np.asarray(Wv2, np.float32)[:, sl].reshape(CT, 128, CL).astype(_BF),
            "wo1": np.asarray(Wo1, np.float32)[:, sl].reshape(CT, 128, CL).astype(_BF),
            "wo2": np.asarray(Wo2, np.float32)[:, sl].reshape(CT, 128, CL).astype(_BF),
            "bq": np.asarray(bq, np.float32)[sl].reshape(2, 128, 1),
            "bk": np.asarray(bk, np.float32)[sl].reshape(2, 128, 1),
            "bv1": np.asarray(bv1, np.float32)[sl].reshape(1, CL).astype(_BF),
            "bv2": np.asarray(bv2, np.float32)[sl].reshape(1, CL).astype(_BF),
            "x1r": (x1f[b, sl, :] + np.asarray(bo1, np.float32)[sl, None]
                    ).reshape(2, 128, N),
            "x2r": (x2f[b, sl, :] + np.asarray(bo2, np.float32)[sl, None]
                    ).reshape(2, 128, N),
        }
        in_maps.append(m)
    return in_maps


def _unshard(res):
    o1 = np.empty((B, C, N), np.float32)
    o2 = np.empty((B, C, N), np.float32)
    for c in range(N_CORES):
        b, hq = c // 2, c % 2
        sl = slice(CL * hq, CL * hq + CL)
        o1[b, sl, :] = res[c]["o1"].reshape(CL, N)
        o2[b, sl, :] = res[c]["o2"].reshape(CL, N)
    shape = (B, C, 8, 16, 16)
    return o1.reshape(shape), o2.reshape(shape)


def kernel(**inputs):
    in_maps = _in_maps(**inputs)
    nc = _get_nc()
    res = run_bass_kernel_spmd(nc, in_maps, list(range(N_CORES))).results
    return _unshard(res)


# revision 17
# speedup vs baseline: 1.3433x; 1.3433x over previous
"""Trainium2 Bass kernel for the dual-softmax cross-attention module.

Sharding: 8 cores = batch (4) x head-half (2).  Core c handles batch c//2 and
heads 4*(c%2) .. 4*(c%2)+4.  Each core computes Q/K/V projections for its
head-group, the 2048x2048 score matrix per head, one shared E = exp(s/8)
(both softmaxes are shift-invariant; scores are O(1) so no max subtraction),
contexts for both streams, exchanges context halves with its pair core via a
2-core AllGather, and produces a disjoint 256-channel slice of both outputs.

All matmuls run in bf16 (fp32 PSUM accumulation); residual + output stay fp32.
"""

import sys

for _p in ("/opt/trn_rl_repo", "/opt/pypackages"):
    if _p not in sys.path:
        sys.path.insert(0, _p)

import numpy as np
import ml_dtypes

import concourse.bass as bass
import concourse.tile as tile
from concourse import bacc, mybir
from concourse.bass_utils import run_bass_kernel_spmd

F32 = mybir.dt.float32
BF16 = mybir.dt.bfloat16
AF = mybir.ActivationFunctionType
AX = mybir.AxisListType

N_CORES = 8
B = 4          # batch
C = 512        # channels
N = 2048       # tokens (8*16*16)
H = 8          # heads
DH = 64        # head dim
HL = 4         # heads per core
CL = 256       # channels per core (head-group)
NT = N // 128  # 16 token tiles
CT = C // 128  # 4 channel tiles

_BF = ml_dtypes.bfloat16


def _build():
    nc = bacc.Bacc("TRN2", target_bir_lowering=False, debug=False,
                   num_devices=N_CORES)

    def din(name, shape, dt=BF16):
        return nc.dram_tensor(name, shape, dt, kind="ExternalInput").ap()

    x1b = din("x1b", [CT, 128, N])          # x1[b] channel-major, bf16
    x2b = din("x2b", [CT, 128, N])
    wq = din("wq", [128, CT, CL])           # column slice of Wq, pre-permuted
    wk = din("wk", [128, CT, CL])
    wv1 = din("wv1", [128, CT, CL])
    wv2 = din("wv2", [128, CT, CL])
    wo1 = din("wo1", [128, CT, CL])         # Wo columns for my output rows
    wo2 = din("wo2", [128, CT, CL])
    bq = din("bq", [128, 2, 1], F32)        # bias slices per M-tile
    bk = din("bk", [128, 2, 1], F32)
    bv1 = din("bv1", [1, CL])
    bv2 = din("bv2", [1, CL])
    x1r = din("x1r", [2, 128, N], F32)      # x1[b] residual slice + bo1
    x2r = din("x2r", [2, 128, N], F32)

    o1 = nc.dram_tensor("o1", [2, 128, N], F32, kind="ExternalOutput").ap()
    o2 = nc.dram_tensor("o2", [2, 128, N], F32, kind="ExternalOutput").ap()

    with tile.TileContext(nc) as tc:
        _emit(nc, tc, locals())
    nc.compile()
    return nc


def _emit(nc, tc, t):
    x1b, x2b = t["x1b"], t["x2b"]
    wq, wk, wv1, wv2 = t["wq"], t["wk"], t["wv1"], t["wv2"]
    wo1, wo2 = t["wo1"], t["wo2"]
    bq, bk, bv1, bv2 = t["bq"], t["bk"], t["bv1"], t["bv2"]
    x1r, x2r, o1, o2 = t["x1r"], t["x2r"], t["o1"], t["o2"]

    from contextlib import ExitStack
    ctx = ExitStack()
    with ctx:
        persist = ctx.enter_context(tc.tile_pool(name="persist", bufs=1))
        small = ctx.enter_context(tc.tile_pool(name="small", bufs=8))
        vp_pool = ctx.enter_context(tc.tile_pool(name="vp", bufs=4))
        dram = ctx.enter_context(tc.tile_pool(name="dram", bufs=2, space="DRAM"))

        # ---- persistent SBUF tensors ----
        wq_s = persist.tile([128, CT, CL], BF16, tag="wq")
        wk_s = persist.tile([128, CT, CL], BF16, tag="wk")
        wv1_s = persist.tile([128, CT, CL], BF16, tag="wv1")
        wv2_s = persist.tile([128, CT, CL], BF16, tag="wv2")
        wo1_s = persist.tile([128, CT, CL], BF16, tag="wo1")
        wo2_s = persist.tile([128, CT, CL], BF16, tag="wo2")
        bq_s = persist.tile([128, 2, 1], F32, tag="bq")
        bk_s = persist.tile([128, 2, 1], F32, tag="bk")
        bv1_s = persist.tile([1, CL], BF16, tag="bv1")
        bv2_s = persist.tile([1, CL], BF16, tag="bv2")
        ones_s = persist.tile([1, N], BF16, tag="ones")
        onec_s = persist.tile([128, 1], F32, tag="onec")
        qt_s = persist.tile([128, 2, N], BF16, tag="qt")    # Q^T  (chan-major)
        kt_s = persist.tile([128, 2, N], BF16, tag="kt")    # K^T
        v1tok = persist.tile([128, NT, CL], BF16, tag="v1tok")  # token-major V1
        v2tok = persist.tile([128, NT, CL], BF16, tag="v2tok")
        # E^T of one head, qt-major: element (k, q) at [k%128, q//128,
        # (k//128)*128 + q%128] -- transpose DMA writes are contiguous.
        et_buf = persist.tile([128, NT, N], BF16, tag="et")
        ctxm1 = persist.tile([128, CT, N], BF16, tag="ctxm1")   # gathered ctx^T
        ctxm2 = persist.tile([128, CT, N], BF16, tag="ctxm2")

        for dst, src in ((wq_s, wq), (wk_s, wk), (wv1_s, wv1), (wv2_s, wv2),
                         (wo1_s, wo1), (wo2_s, wo2)):
            nc.sync.dma_start(dst[:, :, :], src[:, :, :])
        nc.sync.dma_start(bq_s[:, :, :], bq[:, :, :])
        nc.sync.dma_start(bk_s[:, :, :], bk[:, :, :])
        nc.sync.dma_start(bv1_s[:, :], bv1[:, :])
        nc.sync.dma_start(bv2_s[:, :], bv2[:, :])
        nc.vector.memset(ones_s[:, :], 1.0)
        nc.vector.memset(onec_s[:, :], 1.0)

        # ---- P1: projections ----
        # x1 tiles -> Q (chan-major) and V1 (token-major); then x2 -> K, V2.
        p1 = ExitStack()
        pj_ps = p1.enter_context(tc.tile_pool(name="pj_ps", bufs=2, space="PSUM"))
        xb_pool = p1.enter_context(tc.tile_pool(name="xb", bufs=4))
        for xb_dram, w_qk, b_qk, qk_dst, w_v, b_v, v_dst in (
            (x1b, wq_s, bq_s, qt_s, wv1_s, bv1_s, v1tok),
            (x2b, wk_s, bk_s, kt_s, wv2_s, bv2_s, v2tok),
        ):
            xt = [xb_pool.tile([128, N], BF16, tag="xb", name=f"xt{i}")
                  for i in range(CT)]
            for ti in range(CT):
                nc.sync.dma_start(xt[ti][:, :], xb_dram[ti, :, :])
            # chan-major Q/K:  out[cl, n] = sum_cin W[cin, cl] * x[cin, n]
            for m in range(2):
                ps = pj_ps.tile([128, N], F32, tag="pj")
                for ch in range(4):
                    for ti in range(CT):
                        nc.tensor.matmul(
                            ps[:, ch * 512:(ch + 1) * 512],
                            w_qk[:, ti, m * 128:(m + 1) * 128],
                            xt[ti][:, ch * 512:(ch + 1) * 512],
                            start=(ti == 0), stop=(ti == CT - 1))
                nc.scalar.activation(qk_dst[:, m, :], ps[:, :], AF.Identity,
                                     bias=b_qk[:, m, :])
            # token-major V:  out[n, cl] = sum_cin x[cin, n] * W[cin, cl] + bv
            for nt in range(NT):
                ps = pj_ps.tile([128, CL], F32, tag="pj")
                for ti in range(CT):
                    nc.tensor.matmul(
                        ps[:, :], xt[ti][:, nt * 128:(nt + 1) * 128],
                        w_v[:, ti, :], start=(ti == 0), stop=False)
                nc.tensor.matmul(ps[:, :], ones_s[:, nt * 128:(nt + 1) * 128],
                                 b_v[:, :], start=False, stop=True)
                nc.scalar.activation(v_dst[:, nt, :], ps[:, :], AF.Copy)
        p1.close()

        # ---- P2: per-head attention ----
        # Per head: stream qtiles (scores -> exp(+rowsum) -> ctx2(+colsum row)
        # -> transpose), then convert the colsum row to a column via 16 K=1
        # matmuls, then ctx1 over E^T.  ctx2 uses a ones-augmented lhsT so
        # PSUM row 64 accumulates colsum for free.
        p2 = ExitStack()
        sc_ps = p2.enter_context(tc.tile_pool(name="sc_ps", bufs=2, space="PSUM"))
        ctx_ps = p2.enter_context(tc.tile_pool(name="ctx_ps", bufs=1, space="PSUM"))
        eslab = p2.enter_context(tc.tile_pool(name="eslab", bufs=6))
        gsrc_pool = p2.enter_context(tc.tile_pool(name="gsrc", bufs=2))
        csrow_pool = p2.enter_context(tc.tile_pool(name="csrow", bufs=1))
        for hl in range(HL):
            g, poff = hl // 2, 64 * (hl % 2)
            q_l = qt_s[poff:poff + 64, g, :]
            k_l = kt_s[poff:poff + 64, g, :]
            cps_a = ctx_ps.tile([128, N], F32, tag="ctx")  # ctx2 0:64, colsum row 64
            for qt in range(NT):
                es = eslab.tile([128, N], BF16, tag="es")
                rs_p = small.tile([128, 2], F32, tag="rsp")
                for u in range(2):
                    ps = sc_ps.tile([128, 1024], F32, tag="sc", name=f"sps{u}")
                    for ch in range(2):
                        off = u * 1024 + ch * 512
                        nc.tensor.matmul(
                            ps[:, ch * 512:(ch + 1) * 512],
                            q_l[:, qt * 128:(qt + 1) * 128],
                            k_l[:, off:off + 512],
                            start=True, stop=True)
                    nc.scalar.activation(es[:, u * 1024:(u + 1) * 1024], ps[:, :],
                                         AF.Exp, scale=0.125,
                                         accum_out=rs_p[:, u:u + 1])
                rs = small.tile([128, 1], F32, tag="rs")
                nc.vector.tensor_add(rs[:, :], rs_p[:, 0:1], rs_p[:, 1:2])
                rr = small.tile([128, 1], F32, tag="rr")
                nc.vector.reciprocal(rr[:, :], rs[:, :])
                v2p = vp_pool.tile([128, DH + 1], BF16, tag="v2p")
                nc.vector.tensor_scalar_mul(
                    v2p[:, 0:DH], v2tok[:, qt, hl * DH:(hl + 1) * DH], rr[:, :])
                nc.vector.memset(v2p[:, DH:DH + 1], 1.0)
                for ch in range(4):
                    nc.tensor.matmul(
                        cps_a[0:DH + 1, ch * 512:(ch + 1) * 512],
                        v2p[:, :], es[:, ch * 512:(ch + 1) * 512],
                        start=(qt == 0), stop=(qt == NT - 1))
                # transpose E-slab into E^T buffer (contiguous writes), two
                # k-halves so ctx1 of the previous head releases et_buf early
                for u in range(2):
                    nc.sync.dma_start(
                        et_buf[:, qt, u * 1024:(u + 1) * 1024].rearrange(
                            "p (a b) -> p a b", b=128),
                        es[:, u * 1024:(u + 1) * 1024], transpose=True)
            # ctx2 + colsum row done: evacuate ctx2, build colsum column
            gs = gsrc_pool.tile([128, N], BF16, tag="gs")
            nc.vector.tensor_copy(gs[0:64, :], cps_a[0:64, :])
            csrow = csrow_pool.tile([65, N], F32, tag="csr")
            nc.scalar.activation(csrow[64:65, :], cps_a[64:65, :], AF.Copy)
            cs_ps = sc_ps.tile([128, 1024], F32, tag="sc", name="cs_ps")
            for kt in range(NT):
                nc.tensor.matmul(cs_ps[:, kt:kt + 1],
                                 csrow[64:65, kt * 128:(kt + 1) * 128],
                                 onec_s[64:65, :], start=True, stop=True)
            cr_t = small.tile([128, NT], F32, tag="cr")
            nc.vector.reciprocal(cr_t[:, :], cs_ps[:, 0:NT])
            # ctx1 over E^T (rows 64:128 of a fresh ctx psum tile)
            cps_b = ctx_ps.tile([128, N], F32, tag="ctx", name="cps_b")
            for kt in range(NT):
                v1p = vp_pool.tile([128, DH], BF16, tag="v1p")
                nc.vector.tensor_scalar_mul(
                    v1p[:, :], v1tok[:, kt, hl * DH:(hl + 1) * DH],
                    cr_t[:, kt:kt + 1])
                for ch in range(4):
                    nc.tensor.matmul(
                        cps_b[64:128, ch * 512:(ch + 1) * 512],
                        v1p[:, :],
                        et_buf[:, 4 * ch:4 * (ch + 1), kt * 128:(kt + 1) * 128],
                        start=(kt == 0), stop=(kt == NT - 1))
            nc.vector.tensor_copy(gs[64:128, :], cps_b[64:128, :])
            # gather the head pair across the 2 cores
            gin = dram.tile([128, N], BF16, tag="gin")
            gout = dram.tile([2, 128, N], BF16, tag="gout")
            nc.sync.dma_start(gin[:, :], gs[:, :])
            nc.gpsimd.collective_compute(
                "AllGather", mybir.AluOpType.bypass,
                replica_groups=[[0, 1], [2, 3], [4, 5], [6, 7]],
                ins=[gin.opt()], outs=[gout.opt()])
            for r in range(2):
                tt = 2 * r + hl // 2
                nc.sync.dma_start(ctxm2[poff:poff + 64, tt, :], gout[r, 0:64, :])
                nc.sync.dma_start(ctxm1[poff:poff + 64, tt, :], gout[r, 64:128, :])

        p2.close()

        # ---- P3: output projections + residual ----
        p3 = ExitStack()
        o_ps = p3.enter_context(tc.tile_pool(name="o_ps", bufs=2, space="PSUM"))
        xr_pool = p3.enter_context(tc.tile_pool(name="xr", bufs=2))
        out_pool = p3.enter_context(tc.tile_pool(name="outp", bufs=2))
        for w_s, cm, xr, oo in ((wo1_s, ctxm1, x1r, o1), (wo2_s, ctxm2, x2r, o2)):
            for m in range(2):
                xr_t = xr_pool.tile([128, N], F32, tag="xr")
                nc.sync.dma_start(xr_t[:, :], xr[m, :, :])
                ps = o_ps.tile([128, N], F32, tag="o")
                for ch in range(4):
                    for ti in range(CT):
                        nc.tensor.matmul(
                            ps[:, ch * 512:(ch + 1) * 512],
                            w_s[:, ti, m * 128:(m + 1) * 128],
                            cm[:, ti, ch * 512:(ch + 1) * 512],
                            start=(ti == 0), stop=(ti == CT - 1))
                ot = out_pool.tile([128, N], F32, tag="ot")
                nc.vector.tensor_add(ot[:, :], ps[:, :], xr_t[:, :])
                nc.sync.dma_start(oo[m, :, :], ot[:, :])
        p3.close()


_NC_CACHE = None


def _get_nc():
    global _NC_CACHE
    if _NC_CACHE is None:
        _NC_CACHE = _build()
    return _NC_CACHE


def _in_maps(x1, x2, Wq, bq, Wk, bk, Wv1, bv1, Wv2, bv2, Wo1, bo1, Wo2, bo2):
    x1f = np.asarray(x1, np.float32).reshape(B, C, N)
    x2f = np.asarray(x2, np.float32).reshape(B, C, N)
    in_maps = []
    for c in range(N_CORES):
        b, hq = c // 2, c % 2
        sl = slice(CL * hq, CL * hq + CL)
        def wslice(W):
            return np.ascontiguousarray(
                np.asarray(W, np.float32)[:, sl].reshape(CT, 128, CL)
                .transpose(1, 0, 2)).astype(_BF)

        m = {
            "x1b": x1f[b].reshape(CT, 128, N).astype(_BF),
            "x2b": x2f[b].reshape(CT, 128, N).astype(_BF),
            "wq": wslice(Wq), "wk": wslice(Wk),
            "wv1": wslice(Wv1), "wv2": wslice(Wv2),
            "wo1": wslice(Wo1), "wo2": wslice(Wo2),
            "bq": np.ascontiguousarray(
                np.asarray(bq, np.float32)[sl].reshape(2, 128).T).reshape(128, 2, 1),
            "bk": np.ascontiguousarray(
                np.asarray(bk, np.float32)[sl].reshape(2, 128).T).reshape(128, 2, 1),
            "bv1": np.asarray(bv1, np.float32)[sl].reshape(1, CL).astype(_BF),
            "bv2": np.asarray(bv2, np.float32)[sl].reshape(1, CL).astype(_BF),
            "x1r": (x1f[b, sl, :] + np.asarray(bo1, np.float32)[sl, None]
                    ).reshape(2, 128, N),
            "x2r": (x2f[b, sl, :] + np.asarray(bo2, np.float32)[sl, None]
                    ).reshape(2, 128, N),
        }
        in_maps.append(m)
    return in_maps


def _unshard(res):
    o1 = np.empty((B, C, N), np.float32)
    o2 = np.empty((B, C, N), np.float32)
    for c in range(N_CORES):
        b, hq = c // 2, c % 2
        sl = slice(CL * hq, CL * hq + CL)
        o1[b, sl, :] = res[c]["o1"].reshape(CL, N)
        o2[b, sl, :] = res[c]["o2"].reshape(CL, N)
    shape = (B, C, 8, 16, 16)
    return o1.reshape(shape), o2.reshape(shape)


def kernel(**inputs):
    in_maps = _in_maps(**inputs)
    nc = _get_nc()
    res = run_bass_kernel_spmd(nc, in_maps, list(range(N_CORES))).results
    return _unshard(res)
